# revision 1
# baseline (speedup 1.0000x reference)
"""KMeansProbSampler Trainium2 kernel (8-core SPMD).

Algorithm (per reference): 8 iterations of
  d2[p,c]   = (h_p - a_c)^2 + (w_p - b_c)^2        (pixel grid 1024x1024, C=128)
  assign[p] = argmin_c max(1, sqrt(d2))            (first-index tie break)
  new[c]    = sum_{p: assign==c} coords_p * heatmap_p / max(1, sqrt(min d2))

Mapping:
  - Shard pixel rows across 8 cores (128 rows each). A "tile" is one image
    column within the shard: 128 pixels on SBUF partitions.
  - d2 for a tile x all 128 clusters via one K=4 PE matmul using
    block-recentered coordinates: lhsT rows [h', w', 1, h'^2+w'^2] (host
    precomputed, streamed from DRAM per 128-column block), rhs rows
    [-2a', -2b', a'^2+b'^2 (+dup mask), 1] built on device each iteration.
    Recentring (h-512, w - block_center) keeps the expanded form's
    cancellation error small at small d2.
  - argmin is replaced by value-matching: m2 = min_c d2 (DVE segmented
    reduce), scaled-one-hot = (d2 == m2) * (1/max(1,sqrt(m2))) in a single
    DVE tensor_scalar op. Duplicate clusters (empty clusters collapse to
    (0,0) from iteration 2 on) get +1e30 in the rhs norm row, computed on
    device, so the first duplicate wins exactly like jnp.argmin.
  - scatter: PE matmul acc[c, 0:2] += soh^T @ [h*hm, w*hm] (host precomputed
    moving operand, N=2), PSUM-accumulated over all 1024 tiles.
  - per-iteration AllReduce of the [128, 2] partial sums across 8 cores.
"""

import os
import sys

import numpy as np

H = 1024
W = 1024
C = 128
N_ITER = 8
NCORES = 8
RPC = H // NCORES  # rows per core
P = 128            # partitions = pixels per tile
NT = W             # tiles (columns) per core
TPB = 128          # tiles per w-block
WG = 4             # tiles per PSUM group ([128, 512] = one bank)
GPB = 4            # groups per sqrt/recip batch (16 tiles)
S_H = 512.0        # global h recentering
BIG = 1.0e30       # duplicate-cluster mask

_REPO_CANDIDATES = ("/opt/trn_rl_repo", "/root/.axon_site/_ro/trn_rl_repo")


def _ensure_repo():
    try:
        import concourse  # noqa: F401
        return
    except ImportError:
        pass
    for p in _REPO_CANDIDATES:
        if os.path.isdir(p):
            sys.path.insert(0, p)
            break
    import concourse  # noqa: F401


def build_nc(n_iter: int = N_ITER, nt: int = NT, ncores: int = NCORES):
    """Build the SPMD Bass program (same program for every core)."""
    _ensure_repo()
    import concourse.bacc as bacc
    import concourse.mybir as mybir
    import concourse.tile as tile

    f32 = mybir.dt.float32
    Alu = mybir.AluOpType
    Act = mybir.ActivationFunctionType
    X = mybir.AxisListType.X

    nblk = (nt + TPB - 1) // TPB
    assert nt % TPB == 0

    nc = bacc.Bacc(
        "TRN2",
        target_bir_lowering=False,
        debug=False,
        num_devices=ncores,
    )

    # ---- I/O ----
    pixT_d = nc.dram_tensor("pixT", [4 * nblk, TPB * P], f32, kind="ExternalInput")
    vhw_d = nc.dram_tensor("vhw", [P, 2 * nt], f32, kind="ExternalInput")
    ext0_d = nc.dram_tensor("ext0", [4 * nblk, C], f32, kind="ExternalInput")
    ident_d = nc.dram_tensor("ident", [P, P], f32, kind="ExternalInput")
    ltri_d = nc.dram_tensor("ltri", [P, P], f32, kind="ExternalInput")
    out_d = nc.dram_tensor("out", [C, 2], f32, kind="ExternalOutput")

    with tile.TileContext(nc) as tc:
        from contextlib import ExitStack

        with ExitStack() as st:
            const = st.enter_context(tc.tile_pool(name="const", bufs=1))
            stpool = st.enter_context(tc.tile_pool(name="stage", bufs=2))
            spool = st.enter_context(tc.tile_pool(name="s", bufs=10))
            mpool = st.enter_context(tc.tile_pool(name="m2", bufs=3))
            sohp = st.enter_context(tc.tile_pool(name="soh", bufs=8))
            smal = st.enter_context(tc.tile_pool(name="small", bufs=4))
            eqp = st.enter_context(tc.tile_pool(name="eq", bufs=2))
            psd = st.enter_context(tc.tile_pool(name="psd", bufs=3, space="PSUM"))
            psa = st.enter_context(tc.tile_pool(name="psa", bufs=1, space="PSUM"))
            pse = st.enter_context(tc.tile_pool(name="pse", bufs=2, space="PSUM"))
            dram = st.enter_context(tc.tile_pool(name="dram", bufs=2, space="DRAM"))

            # ---- persistent SBUF state ----
            vhw = const.tile([P, 2 * nt], f32)
            ident = const.tile([P, P], f32)
            ltri = const.tile([P, P], f32)
            exts = [const.tile([4, C], f32, name=f"ext{b}", tag=f"ext{b}")
                    for b in range(nblk)]

            nc.gpsimd.dma_start(vhw[:], vhw_d[:])
            nc.gpsimd.dma_start(ident[:], ident_d[:])
            nc.gpsimd.dma_start(ltri[:], ltri_d[:])
            for b in range(nblk):
                nc.gpsimd.dma_start(exts[b][:], ext0_d[4 * b:4 * b + 4, :])

            arout_prev = None
            for it in range(n_iter):
                acc = psa.tile([C, 2], f32, space="PSUM")
                for b in range(nblk):
                    stage = stpool.tile([4, TPB * P], f32, tag="stage")
                    nc.gpsimd.dma_start(stage[:], pixT_d[4 * b:4 * b + 4, :])
                    for gl in range(TPB // WG):
                        t0 = b * TPB + gl * WG
                        psum_d = psd.tile([P, WG * P], f32, space="PSUM")
                        for tau in range(WG):
                            loc = gl * WG + tau
                            nc.tensor.matmul(
                                out=psum_d[:, tau * P:(tau + 1) * P],
                                lhsT=stage[:, loc * P:(loc + 1) * P],
                                rhs=exts[b][:],
                                start=(tau == 0),
                                stop=(tau == WG - 1),
                            )
                        s = spool.tile([P, WG * P], f32)
                        nc.scalar.copy(out=s[:], in_=psum_d[:])

                        gg = gl % GPB
                        if gg == 0:
                            m2 = mpool.tile([P, GPB * WG], f32, tag="m2")
                            batch_s = []
                            batch_t0 = t0
                        batch_s.append(s)
                        nc.vector.tensor_reduce(
                            out=m2[:, gg * WG:(gg + 1) * WG],
                            in_=s[:].rearrange("p (n x) -> p n x", x=P),
                            axis=X,
                            op=Alu.min,
                        )
                        if gg == GPB - 1:
                            # batched 1/max(1, sqrt(m2)) for these 16 tiles
                            rec = mpool.tile([P, GPB * WG], f32, tag="rec")
                            sq = mpool.tile([P, GPB * WG], f32, tag="sq")
                            nc.vector.tensor_scalar(
                                out=sq[:], in0=m2[:], scalar1=1.0, scalar2=None,
                                op0=Alu.max,
                            )
                            nc.scalar.activation(out=sq[:], in_=sq[:],
                                                 func=Act.Sqrt)
                            nc.vector.reciprocal(out=rec[:], in_=sq[:])
                            for q, s_q in enumerate(batch_s):
                                for tau in range(WG):
                                    t = batch_t0 + q * WG + tau
                                    col = q * WG + tau
                                    soh = sohp.tile([P, P], f32)
                                    nc.vector.tensor_scalar(
                                        out=soh[:],
                                        in0=s_q[:, tau * P:(tau + 1) * P],
                                        scalar1=m2[:, col:col + 1],
                                        scalar2=rec[:, col:col + 1],
                                        op0=Alu.is_equal,
                                        op1=Alu.mult,
                                    )
                                    nc.tensor.matmul(
                                        out=acc[:],
                                        lhsT=soh[:],
                                        rhs=vhw[:, 2 * t:2 * t + 2],
                                        start=(t == 0),
                                        stop=(t == nt - 1),
                                    )

                # ---- partial [C,2] -> AllReduce ----
                part = smal.tile([C, 2], f32, tag="part")
                nc.scalar.copy(out=part[:], in_=acc[:])
                arin = dram.tile([C, 2], f32)
                arout = dram.tile([C, 2], f32)
                nc.gpsimd.dma_start(arin[:], part[:])
                nc.gpsimd.collective_compute(
                    "AllReduce",
                    Alu.add,
                    replica_groups=[list(range(ncores))],
                    ins=[arin[:].opt()],
                    outs=[arout[:].opt()],
                )
                arout_prev = arout

                if it == n_iter - 1:
                    break

                # ---- epilogue: rebuild per-block ext from reduced clusters ----
                ncs = smal.tile([C, 2], f32, tag="ncs")
                nc.gpsimd.dma_start(ncs[:], arout[:])

                # broadcast a and b along free dim: bc[i, j] = coord_j
                abc = pse.tile([C, C], f32, space="PSUM", tag="bc")
                nc.tensor.transpose(
                    out=abc[:], in_=ncs[:, 0:1].to_broadcast([C, C]),
                    identity=ident[:],
                )
                eqa = eqp.tile([C, C], f32, tag="eqa")
                nc.vector.tensor_scalar(
                    out=eqa[:], in0=abc[:], scalar1=ncs[:, 0:1], scalar2=None,
                    op0=Alu.is_equal,
                )
                bbc = pse.tile([C, C], f32, space="PSUM", tag="bc")
                nc.tensor.transpose(
                    out=bbc[:], in_=ncs[:, 1:2].to_broadcast([C, C]),
                    identity=ident[:],
                )
                eqb = eqp.tile([C, C], f32, tag="eqb")
                nc.vector.tensor_scalar(
                    out=eqb[:], in0=bbc[:], scalar1=ncs[:, 1:2], scalar2=None,
                    op0=Alu.is_equal,
                )
                nc.vector.tensor_tensor(out=eqa[:], in0=eqa[:], in1=eqb[:],
                                        op=Alu.mult)
                nc.vector.tensor_tensor(out=eqa[:], in0=eqa[:], in1=ltri[:],
                                        op=Alu.mult)
                # cf[i] = count of earlier duplicates of cluster i
                cfs = smal.tile([C, 1], f32, tag="cfs")
                nc.vector.tensor_reduce(out=cfs[:], in_=eqa[:], axis=X,
                                        op=Alu.add)
                nc.vector.tensor_scalar(
                    out=cfs[:], in0=cfs[:], scalar1=BIG, scalar2=None,
                    op0=Alu.mult,
                )
                apc = smal.tile([C, 1], f32, tag="apc")
                nc.vector.tensor_scalar(out=apc[:], in0=ncs[:, 0:1],
                                        scalar1=S_H, scalar2=None,
                                        op0=Alu.subtract)
                basec = smal.tile([C, 1], f32, tag="basec")
                nc.vector.tensor_tensor(out=basec[:], in0=apc[:], in1=apc[:],
                                        op=Alu.mult)
                nc.vector.tensor_tensor(out=basec[:], in0=basec[:], in1=cfs[:],
                                        op=Alu.add)
                bpc = smal.tile([C, 1], f32, tag="bpc")
                b2c = smal.tile([C, 1], f32, tag="b2c")
                for b in range(nblk):
                    w0 = float(b * TPB + 64)
                    extt = eqp.tile([C, 4], f32, tag="extt")
                    nc.vector.tensor_scalar(
                        out=extt[:, 0:1], in0=ncs[:, 0:1], scalar1=S_H,
                        scalar2=-2.0, op0=Alu.subtract, op1=Alu.mult,
                    )
                    nc.vector.tensor_scalar(
                        out=extt[:, 1:2], in0=ncs[:, 1:2], scalar1=w0,
                        scalar2=-2.0, op0=Alu.subtract, op1=Alu.mult,
                    )
                    nc.vector.tensor_scalar(
                        out=bpc[:], in0=ncs[:, 1:2], scalar1=w0, scalar2=None,
                        op0=Alu.subtract,
                    )
                    nc.vector.tensor_tensor(out=b2c[:], in0=bpc[:], in1=bpc[:],
                                            op=Alu.mult)
                    nc.vector.tensor_tensor(out=extt[:, 2:3], in0=b2c[:],
                                            in1=basec[:], op=Alu.add)
                    nc.vector.memset(extt[:, 3:4], 1.0)
                    extp = pse.tile([4, C], f32, space="PSUM", tag="extp")
                    nc.tensor.transpose(out=extp[:], in_=extt[:],
                                        identity=ident[:])
                    nc.scalar.copy(out=exts[b][:], in_=extp[:])

            # final output
            nc.gpsimd.dma_start(out_d[:], arout_prev[:])

    nc.compile()
    return nc


def make_core_inputs(core: int, clusters: np.ndarray, heatmap: np.ndarray,
                     nt: int = NT):
    """Host-precomputed per-core constant tables."""
    nblk = nt // TPB
    r0 = core * RPC
    hs = (np.arange(P, dtype=np.float32) + np.float32(r0))
    hp = (hs - np.float32(S_H)).astype(np.float32)

    pixT = np.zeros((4 * nblk, TPB * P), np.float32)
    for b in range(nblk):
        w0 = np.float32(b * TPB + 64)
        for tau in range(TPB):
            t = b * TPB + tau
            wp = np.float32(np.float32(t) - w0)
            sl = slice(tau * P, (tau + 1) * P)
            pixT[4 * b + 0, sl] = hp
            pixT[4 * b + 1, sl] = wp
            pixT[4 * b + 2, sl] = 1.0
            pixT[4 * b + 3, sl] = (hp * hp + wp * wp).astype(np.float32)

    hm = heatmap[r0:r0 + RPC, :nt].astype(np.float32)
    vhw = np.empty((P, 2 * nt), np.float32)
    vhw[:, 0::2] = (hs[:, None] * hm).astype(np.float32)
    vhw[:, 1::2] = (np.arange(nt, dtype=np.float32)[None, :] * hm).astype(np.float32)

    a = clusters[:, 0].astype(np.float32)
    b_ = clusters[:, 1].astype(np.float32)
    ext0 = np.zeros((4 * nblk, C), np.float32)
    ap = (a - np.float32(S_H)).astype(np.float32)
    for b in range(nblk):
        w0 = np.float32(b * TPB + 64)
        bp = (b_ - w0).astype(np.float32)
        ext0[4 * b + 0] = (np.float32(-2.0) * ap).astype(np.float32)
        ext0[4 * b + 1] = (np.float32(-2.0) * bp).astype(np.float32)
        ext0[4 * b + 2] = (ap * ap + bp * bp).astype(np.float32)
        ext0[4 * b + 3] = 1.0

    return {
        "pixT": pixT,
        "vhw": vhw,
        "ext0": ext0,
        "ident": np.eye(P, dtype=np.float32),
        "ltri": np.tril(np.ones((P, P), np.float32), -1),
    }


_NC_CACHE = {}


def kernel(clusters: np.ndarray, heatmap: np.ndarray) -> np.ndarray:
    _ensure_repo()
    from concourse.bass_utils import run_bass_kernel_spmd

    clusters = np.asarray(clusters, np.float32)
    heatmap = np.asarray(heatmap, np.float32)

    key = (N_ITER, NT)
    if key not in _NC_CACHE:
        _NC_CACHE[key] = build_nc()
    nc = _NC_CACHE[key]

    in_maps = [make_core_inputs(k, clusters, heatmap) for k in range(NCORES)]
    res = run_bass_kernel_spmd(nc, in_maps, list(range(NCORES)))
    return np.asarray(res.results[0]["out"], np.float32)


if __name__ == "__main__":
    _ensure_repo()
    nc = build_nc(n_iter=int(sys.argv[1]) if len(sys.argv) > 1 else 1,
                  nt=int(sys.argv[2]) if len(sys.argv) > 2 else 128)
    print("built + compiled OK")



# revision 5
# speedup vs baseline: 1.2349x; 1.2349x over previous
"""KMeansProbSampler Trainium2 kernel (8-core SPMD), v2.

Algorithm (per reference): 8 iterations of
  d2[p,c]   = (h_p - a_c)^2 + (w_p - b_c)^2        (pixel grid 1024x1024, C=128)
  assign[p] = argmin_c max(1, sqrt(d2))            (first-index tie break)
  new[c]    = sum_{p: assign==c} coords_p * heatmap_p / max(1, sqrt(min d2))

Mapping (per core: 128 pixel rows, 1024 columns; tile = one column):
  - dist: split-bf16 expansion, K=8 per tile, 4 tiles packed per matmul
    (K=32, N=512, block-diagonal ext) at 1 cyc/row.  Pixel features
    [h', h', w', w', hw2_hi, hw2_lo, 1, 1] (h' per-core, w' per-block
    recentered; all exactly bf16) x cluster features
    [-2a'_hi, -2a'_lo, -2b'_hi, -2b'_lo, 1, 1, s2_hi, s2_lo]
    (hi/lo bf16 splits; d2 error ~1e-1).
  - min/one-hot on DVE in fp16: ACT copies PSUM d2 -> fp16 with scale 2^-5
    (max d2 = 2*1023^2 -> 65408 < fp16 max), tensor_reduce min, then
    one-hot = (d2==m2) * rec in one fused tensor_scalar (16-bit 4x modes).
  - rec = 1/max(1, sqrt(d2)) = reciprocal(sqrt(max(1, 32*m2_scaled))),
    batched over 16 tiles.
  - scatter: acc[2, c] += vhw_t^T @ soh_t: vhw [128, 2] fp16 stationary
    (cheap LDW), one-hot streamed N=128, accumulated over all 1024 tiles.
  - per-iteration AllReduce of [2, 128] partials; cluster state transposed
    back to [C, 2]; ext tables rebuilt on device (duplicate clusters get
    +1e30 -> fp16 inf, so the first duplicate wins like jnp.argmin).
"""

import os
import sys

import numpy as np

H = 1024
W = 1024
C = 128
N_ITER = 8
NCORES = 8
RPC = H // NCORES  # rows per core
P = 128            # partitions = pixels per tile
NT = W             # tiles (columns) per core
GT = 4             # tiles per matmul group
TPB = 128          # tiles per w-block
CHUNK = 16         # groups per pixT DMA chunk
SCALE = 2.0 ** -5  # d2 -> fp16 scale
BIG = 1.0e30       # duplicate-cluster mask

_REPO_CANDIDATES = ("/opt/trn_rl_repo", "/root/.axon_site/_ro/trn_rl_repo")


def _ensure_repo():
    try:
        import concourse  # noqa: F401
        return
    except ImportError:
        pass
    for p in _REPO_CANDIDATES:
        if os.path.isdir(p):
            sys.path.insert(0, p)
            break
    import concourse  # noqa: F401


def build_nc(n_iter: int = N_ITER, nt: int = NT, ncores: int = NCORES):
    """Build the SPMD Bass program (same program for every core)."""
    _ensure_repo()
    import concourse.bacc as bacc
    import concourse.mybir as mybir
    import concourse.tile as tile

    f32 = mybir.dt.float32
    f16 = mybir.dt.float16
    bf16 = mybir.dt.bfloat16
    Alu = mybir.AluOpType
    Act = mybir.ActivationFunctionType
    X = mybir.AxisListType.X

    ng = nt // GT                  # matmul groups
    nblk = (nt + TPB - 1) // TPB   # w-blocks
    assert nt % TPB == 0 and ng % 4 == 0

    nc = bacc.Bacc(
        "TRN2",
        target_bir_lowering=False,
        debug=False,
        num_devices=ncores,
    )

    # ---- I/O ----
    pixT_d = nc.dram_tensor("pixT", [32, ng * P], bf16, kind="ExternalInput")
    vhwT_d = nc.dram_tensor("vhwT", [P, 2 * nt], f16, kind="ExternalInput")
    clus_d = nc.dram_tensor("clus", [C, 2], f32, kind="ExternalInput")
    chv_d = nc.dram_tensor("chv", [C, 1], f32, kind="ExternalInput")
    wB_d = nc.dram_tensor("wB", [C, nblk], f32, kind="ExternalInput")
    ident_d = nc.dram_tensor("ident", [P, P], f32, kind="ExternalInput")
    ltri_d = nc.dram_tensor("ltri", [P, P], f32, kind="ExternalInput")
    out_d = nc.dram_tensor("out", [C, 2], f32, kind="ExternalOutput")

    with tile.TileContext(nc) as tc:
        from contextlib import ExitStack

        with ExitStack() as st:
            const = st.enter_context(tc.tile_pool(name="const", bufs=1))
            stpool = st.enter_context(tc.tile_pool(name="stage", bufs=2))
            d16p = st.enter_context(tc.tile_pool(name="d16", bufs=7))
            sohp = st.enter_context(tc.tile_pool(name="soh", bufs=4))
            m2p = st.enter_context(tc.tile_pool(name="m2", bufs=2))
            sqp = st.enter_context(tc.tile_pool(name="sq", bufs=2))
            recp = st.enter_context(tc.tile_pool(name="rec", bufs=2))
            smal = st.enter_context(tc.tile_pool(name="small", bufs=4))
            eqp = st.enter_context(tc.tile_pool(name="eq", bufs=2))
            psd = st.enter_context(tc.tile_pool(name="psd", bufs=3, space="PSUM"))
            psa = st.enter_context(tc.tile_pool(name="psa", bufs=1, space="PSUM"))
            pse = st.enter_context(tc.tile_pool(name="pse", bufs=1, space="PSUM"))
            dram = st.enter_context(tc.tile_pool(name="dram", bufs=2, space="DRAM"))

            # ---- persistent SBUF state ----
            vhwT = const.tile([P, 2 * nt], f16)
            ident = const.tile([P, P], f32)
            ltri = const.tile([P, P], f32)
            chv = const.tile([C, 1], f32)
            wB = const.tile([C, nblk], f32)
            clus0 = const.tile([C, 2], f32)
            exF = const.tile([C, 8 * nblk], f32)
            rhsAll = const.tile([32, 512 * nblk], bf16)

            nc.gpsimd.dma_start(vhwT[:], vhwT_d[:])
            nc.gpsimd.dma_start(ident[:], ident_d[:])
            nc.gpsimd.dma_start(ltri[:], ltri_d[:])
            nc.gpsimd.dma_start(chv[:], chv_d[:])
            nc.gpsimd.dma_start(wB[:], wB_d[:])
            nc.gpsimd.dma_start(clus0[:], clus_d[:])

            nc.vector.memset(rhsAll[:], 0.0)
            for b in range(nblk):
                nc.vector.memset(exF[:, 8 * b + 4:8 * b + 6], 1.0)

            def build_ext(ncs):
                """Emit ops building rhsAll (block-diag bf16 ext) from ncs [C,2] f32."""
                # duplicate-cluster detection (first duplicate wins)
                abc = pse.tile([C, C], f32, space="PSUM", tag="bc")
                nc.tensor.transpose(
                    out=abc[:], in_=ncs[:, 0:1].to_broadcast([C, C]),
                    identity=ident[:],
                )
                eqa = eqp.tile([C, C], f32, tag="eqa")
                nc.vector.tensor_scalar(
                    out=eqa[:], in0=abc[:], scalar1=ncs[:, 0:1], scalar2=None,
                    op0=Alu.is_equal,
                )
                bbc = pse.tile([C, C], f32, space="PSUM", tag="bc2")
                nc.tensor.transpose(
                    out=bbc[:], in_=ncs[:, 1:2].to_broadcast([C, C]),
                    identity=ident[:],
                )
                eqb = eqp.tile([C, C], f32, tag="eqb")
                nc.vector.tensor_scalar(
                    out=eqb[:], in0=bbc[:], scalar1=ncs[:, 1:2], scalar2=None,
                    op0=Alu.is_equal,
                )
                nc.vector.tensor_tensor(out=eqa[:], in0=eqa[:], in1=eqb[:],
                                        op=Alu.mult)
                nc.vector.tensor_tensor(out=eqa[:], in0=eqa[:], in1=ltri[:],
                                        op=Alu.mult)
                cfs = smal.tile([C, 1], f32, tag="cfs")
                nc.vector.tensor_reduce(out=cfs[:], in_=eqa[:], axis=X,
                                        op=Alu.add)

                # a-part: a' = a - ch; -2a' split hi/lo; s2base = a'^2 + BIG*dup
                aP = smal.tile([C, 1], f32, tag="aP")
                nc.vector.tensor_scalar(out=aP[:], in0=ncs[:, 0:1],
                                        scalar1=chv[:, 0:1], scalar2=None,
                                        op0=Alu.subtract)
                m2a = smal.tile([C, 1], f32, tag="m2a")
                nc.vector.tensor_scalar(out=m2a[:], in0=aP[:], scalar1=-2.0,
                                        scalar2=None, op0=Alu.mult)
                m2a_hb = smal.tile([C, 1], bf16, tag="m2a_hb")
                nc.vector.tensor_copy(out=m2a_hb[:], in_=m2a[:])
                m2a_hf = smal.tile([C, 1], f32, tag="m2a_hf")
                nc.vector.tensor_copy(out=m2a_hf[:], in_=m2a_hb[:])
                m2a_lo = smal.tile([C, 1], f32, tag="m2a_lo")
                nc.vector.tensor_tensor(out=m2a_lo[:], in0=m2a[:],
                                        in1=m2a_hf[:], op=Alu.subtract)
                s2b = smal.tile([C, 1], f32, tag="s2b")
                nc.vector.tensor_tensor(out=s2b[:], in0=aP[:], in1=aP[:],
                                        op=Alu.mult)
                nc.vector.tensor_scalar(out=cfs[:], in0=cfs[:], scalar1=BIG,
                                        scalar2=None, op0=Alu.mult)
                nc.vector.tensor_tensor(out=s2b[:], in0=s2b[:], in1=cfs[:],
                                        op=Alu.add)

                # b-part, all blocks at once: [C, nblk]
                bP8 = smal.tile([C, nblk], f32, tag="bP8")
                nc.vector.tensor_tensor(
                    out=bP8[:], in0=ncs[:, 1:2].to_broadcast([C, nblk]),
                    in1=wB[:], op=Alu.subtract)
                m2b8 = smal.tile([C, nblk], f32, tag="m2b8")
                nc.vector.tensor_scalar(out=m2b8[:], in0=bP8[:], scalar1=-2.0,
                                        scalar2=None, op0=Alu.mult)
                m2b8_hb = smal.tile([C, nblk], bf16, tag="m2b8_hb")
                nc.vector.tensor_copy(out=m2b8_hb[:], in_=m2b8[:])
                m2b8_hf = smal.tile([C, nblk], f32, tag="m2b8_hf")
                nc.vector.tensor_copy(out=m2b8_hf[:], in_=m2b8_hb[:])
                m2b8_lo = smal.tile([C, nblk], f32, tag="m2b8_lo")
                nc.vector.tensor_tensor(out=m2b8_lo[:], in0=m2b8[:],
                                        in1=m2b8_hf[:], op=Alu.subtract)
                s28 = smal.tile([C, nblk], f32, tag="s28")
                nc.vector.tensor_tensor(out=s28[:], in0=bP8[:], in1=bP8[:],
                                        op=Alu.mult)
                nc.vector.tensor_tensor(out=s28[:], in0=s28[:],
                                        in1=s2b[:].to_broadcast([C, nblk]),
                                        op=Alu.add)
                s28_hb = smal.tile([C, nblk], bf16, tag="s28_hb")
                nc.vector.tensor_copy(out=s28_hb[:], in_=s28[:])
                s28_hf = smal.tile([C, nblk], f32, tag="s28_hf")
                nc.vector.tensor_copy(out=s28_hf[:], in_=s28_hb[:])
                s28_lo = smal.tile([C, nblk], f32, tag="s28_lo")
                nc.vector.tensor_tensor(out=s28_lo[:], in0=s28[:],
                                        in1=s28_hf[:], op=Alu.subtract)

                # scatter into exF [C, (b, k)] strided views
                exV = exF[:].rearrange("p (b k) -> p b k", k=8)
                nc.vector.tensor_copy(out=exV[:, :, 0:1],
                                      in_=m2a_hf[:].to_broadcast([C, nblk, 1]))
                nc.vector.tensor_copy(out=exV[:, :, 1:2],
                                      in_=m2a_lo[:].to_broadcast([C, nblk, 1]))
                nc.vector.tensor_copy(out=exV[:, :, 2:3],
                                      in_=m2b8_hf[:].unsqueeze(2))
                nc.vector.tensor_copy(out=exV[:, :, 3:4],
                                      in_=m2b8_lo[:].unsqueeze(2))
                nc.vector.tensor_copy(out=exV[:, :, 6:7],
                                      in_=s28_hf[:].unsqueeze(2))
                nc.vector.tensor_copy(out=exV[:, :, 7:8],
                                      in_=s28_lo[:].unsqueeze(2))

                # transpose -> [8*nblk, C], cast bf16, scatter into rhsAll
                extp = pse.tile([8 * nblk, C], f32, space="PSUM", tag="extp")
                nc.tensor.transpose(out=extp[:], in_=exF[:], identity=ident[:])
                extb = smal.tile([8 * nblk, C], bf16, tag="extb")
                nc.scalar.copy(out=extb[:], in_=extp[:])
                for b in range(nblk):
                    for s in range(GT):
                        nc.gpsimd.dma_start(
                            rhsAll[8 * s:8 * s + 8,
                                   512 * b + P * s:512 * b + P * (s + 1)],
                            extb[8 * b:8 * b + 8, :])

            ncs_cur = clus0
            build_ext(clus0)

            for it in range(n_iter):
                acc = psa.tile([2, C], f32, space="PSUM")
                stage = None
                for g in range(ng):
                    blk = (g * GT) // TPB
                    if g % CHUNK == 0:
                        stage = stpool.tile([32, CHUNK * P], bf16, tag="stage")
                        nc.gpsimd.dma_start(
                            stage[:],
                            pixT_d[:, g * P:(g + CHUNK) * P])
                    ps = psd.tile([P, GT * P], f32, space="PSUM")
                    nc.tensor.matmul(
                        out=ps[:],
                        lhsT=stage[:, (g % CHUNK) * P:(g % CHUNK + 1) * P],
                        rhs=rhsAll[:, 512 * blk:512 * (blk + 1)],
                        start=True, stop=True,
                    )
                    d16 = d16p.tile([P, GT * P], f16)
                    nc.scalar.mul(d16[:], ps[:], SCALE)

                    gg = g % 4
                    if gg == 0:
                        m2b = m2p.tile([P, 16], f32, tag="m2b")
                        batch = []
                    nc.vector.tensor_reduce(
                        out=m2b[:, gg * GT:(gg + 1) * GT],
                        in_=d16[:].rearrange("p (n x) -> p n x", x=P),
                        axis=X, op=Alu.min,
                    )
                    batch.append(d16)
                    if gg == 3:
                        sq = sqp.tile([P, 16], f32, tag="sq")
                        nc.vector.tensor_scalar(
                            out=sq[:], in0=m2b[:], scalar1=1.0 / SCALE,
                            scalar2=1.0, op0=Alu.mult, op1=Alu.max,
                        )
                        nc.scalar.activation(out=sq[:], in_=sq[:], func=Act.Sqrt)
                        rec = recp.tile([P, 16], f32, tag="rec")
                        nc.vector.reciprocal(out=rec[:], in_=sq[:])
                        for q, d16q in enumerate(batch):
                            gq = g - 3 + q
                            soh = sohp.tile([P, GT * P], f16)
                            for s in range(GT):
                                col = q * GT + s
                                nc.vector.tensor_scalar(
                                    out=soh[:, s * P:(s + 1) * P],
                                    in0=d16q[:, s * P:(s + 1) * P],
                                    scalar1=m2b[:, col:col + 1],
                                    scalar2=rec[:, col:col + 1],
                                    op0=Alu.is_equal, op1=Alu.mult,
                                )
                                t = gq * GT + s
                                nc.tensor.matmul(
                                    out=acc[:],
                                    lhsT=vhwT[:, 2 * t:2 * t + 2],
                                    rhs=soh[:, s * P:(s + 1) * P],
                                    start=(t == 0), stop=(t == nt - 1),
                                )

                # ---- iteration end: AllReduce [2, C] partials ----
                accS = smal.tile([2, C], f32, tag="accS")
                nc.scalar.copy(out=accS[:], in_=acc[:])
                arin = dram.tile([2, C], f32)
                arout = dram.tile([2, C], f32)
                nc.gpsimd.dma_start(arin[:], accS[:])
                nc.gpsimd.collective_compute(
                    "AllReduce",
                    Alu.add,
                    replica_groups=[list(range(ncores))],
                    ins=[arin[:].opt()],
                    outs=[arout[:].opt()],
                )
                ncs2 = smal.tile([2, C], f32, tag="ncs2")
                nc.gpsimd.dma_start(ncs2[:], arout[:])
                ncsp = pse.tile([C, 2], f32, space="PSUM", tag="ncsp")
                nc.tensor.transpose(out=ncsp[:], in_=ncs2[:],
                                    identity=ident[0:2, 0:2])
                ncs_new = smal.tile([C, 2], f32, tag="ncs_new")
                nc.scalar.copy(out=ncs_new[:], in_=ncsp[:])

                if it == n_iter - 1:
                    nc.gpsimd.dma_start(out_d[:], ncs_new[:])
                else:
                    build_ext(ncs_new)

    nc.compile()
    return nc


def make_core_inputs(core: int, clusters: np.ndarray, heatmap: np.ndarray,
                     nt: int = NT):
    """Host-precomputed per-core constant tables."""
    import ml_dtypes
    bf16 = ml_dtypes.bfloat16

    ng = nt // GT
    nblk = max(1, nt // TPB)
    r0 = core * RPC
    ch = np.float32(r0 + 64)
    hp = (np.arange(P, dtype=np.float32) - np.float32(64.0))
    hw_h2 = hp * hp

    pixT = np.zeros((32, ng * P), np.float32)
    ts = np.arange(nt, dtype=np.float32)
    w0s = (ts // TPB) * TPB + 64.0
    wps = (ts - w0s).astype(np.float32)
    for s in range(GT):
        # tiles t = 4g+s for g in 0..ng
        wp_s = wps[s::GT]                      # [ng]
        cols = pixT[:, :].reshape(32, ng, P)
        cols[8 * s + 0, :, :] = hp[None, :]
        cols[8 * s + 1, :, :] = hp[None, :]
        cols[8 * s + 2, :, :] = wp_s[:, None]
        cols[8 * s + 3, :, :] = wp_s[:, None]
        hw2 = hw_h2[None, :] + (wp_s * wp_s)[:, None]   # [ng, P]
        hi = hw2.astype(bf16).astype(np.float32)
        lo = hw2 - hi
        cols[8 * s + 4, :, :] = hi
        cols[8 * s + 5, :, :] = lo
        cols[8 * s + 6, :, :] = 1.0
        cols[8 * s + 7, :, :] = 1.0
    pixT = pixT.astype(bf16)

    hm = heatmap[r0:r0 + RPC, :nt].astype(np.float32)
    hs = (np.arange(P, dtype=np.float32) + np.float32(r0))
    vhwT = np.empty((P, 2 * nt), np.float32)
    vhwT[:, 0::2] = hs[:, None] * hm
    vhwT[:, 1::2] = np.arange(nt, dtype=np.float32)[None, :] * hm
    vhwT = vhwT.astype(np.float16)

    wB = np.broadcast_to(
        (np.arange(nblk, dtype=np.float32) * TPB + 64.0)[None, :],
        (C, nblk)).copy()

    return {
        "pixT": pixT,
        "vhwT": vhwT,
        "clus": clusters.astype(np.float32),
        "chv": np.full((C, 1), ch, np.float32),
        "wB": wB,
        "ident": np.eye(P, dtype=np.float32),
        "ltri": np.tril(np.ones((P, P), np.float32), -1),
    }


_NC_CACHE = {}


def kernel(clusters: np.ndarray, heatmap: np.ndarray) -> np.ndarray:
    _ensure_repo()
    from concourse.bass_utils import run_bass_kernel_spmd

    clusters = np.asarray(clusters, np.float32)
    heatmap = np.asarray(heatmap, np.float32)

    key = (N_ITER, NT)
    if key not in _NC_CACHE:
        _NC_CACHE[key] = build_nc()
    nc = _NC_CACHE[key]

    in_maps = [make_core_inputs(k, clusters, heatmap) for k in range(NCORES)]
    res = run_bass_kernel_spmd(nc, in_maps, list(range(NCORES)))
    return np.asarray(res.results[0]["out"], np.float32)


if __name__ == "__main__":
    _ensure_repo()
    nc = build_nc(n_iter=int(sys.argv[1]) if len(sys.argv) > 1 else 1,
                  nt=int(sys.argv[2]) if len(sys.argv) > 2 else 128)
    print("built + compiled OK")


# revision 10
# speedup vs baseline: 1.5858x; 1.2842x over previous
"""KMeansProbSampler Trainium2 kernel (8-core SPMD), v3.

Algorithm (per reference): 8 iterations of
  d2[p,c]   = (h_p - a_c)^2 + (w_p - b_c)^2        (pixel grid 1024x1024, C=128)
  assign[p] = argmin_c max(1, sqrt(d2))            (first-index tie break)
  new[c]    = sum_{p: assign==c} coords_p * heatmap_p / max(1, sqrt(min d2))

Mapping (per core: 128 pixel rows, 1024 columns; tile = one column):
  - dist: split-bf16 expansion, K=8 per tile, 4 tiles packed per matmul
    (K=32, N=512, block-diagonal ext) at 1 cyc/row.  Pixel features
    [h', h', w', w', hw2_hi, hw2_lo, 1, 1] (h' per-core, w' per-block
    recentered; all exactly bf16) x cluster features
    [-2a'_hi, -2a'_lo, -2b'_hi, -2b'_lo, 1, 1, s2_hi, s2_lo]
    (hi/lo bf16 splits; d2 error ~1e-1).
  - m2n = -min_c d2 per tile: DVE tensor_reduce(min, negate) straight from
    PSUM fp32 (exact).
  - one-hot: for 14 of each 16 tiles the SCALAR engine computes the
    COMPLEMENT notho = Sign(d2 - m2) in {0,1} (exact fp32 compare via
    per-partition bias); the other 2 tiles use a DVE tensor_scalar
    ((d2 + m2n) is_equal 0) positive one-hot.  This splits the compare
    pass across two engines.
  - rec = 1/max(1, sqrt(m2)) is folded into the scatter weights:
    vs[p, (t,d)] = vhw * rec (one small TT per 16 tiles).
  - scatter: complement tiles accumulate accN[2,c] += vs_t^T @ notho_t and
    a ones-matmul accumulates T[2,(t,d)] = column sums of vs over the same
    tiles; positive tiles accumulate accP.  True sums = accP + T - accN,
    fixed up after the per-iteration AllReduce of the raw [2, 284] strip
    (all three accumulators are linear in the per-core data).
  - duplicate clusters get +1e30 in d2 -> complement 1 everywhere -> their
    fixed-up sum is exactly 0, matching reference empty-cluster behavior.
"""

import os
import sys

import numpy as np

H = 1024
W = 1024
C = 128
N_ITER = 8
NCORES = 8
RPC = H // NCORES  # rows per core
P = 128            # partitions = pixels per tile
NT = W             # tiles (columns) per core
GT = 4             # tiles per matmul group
TPB = 128          # tiles per w-block
CHUNK = 16         # groups per pixT DMA chunk
NDVE = 2           # tiles per 16-slab handled by DVE (slots 16-NDVE..15)
BIG = 1.0e30       # duplicate-cluster mask

_REPO_CANDIDATES = ("/opt/trn_rl_repo", "/root/.axon_site/_ro/trn_rl_repo")


def _ensure_repo():
    try:
        import concourse  # noqa: F401
        return
    except ImportError:
        pass
    for p in _REPO_CANDIDATES:
        if os.path.isdir(p):
            sys.path.insert(0, p)
            break
    import concourse  # noqa: F401


def build_nc(n_iter: int = N_ITER, nt: int = NT, ncores: int = NCORES):
    """Build the SPMD Bass program (same program for every core)."""
    _ensure_repo()
    import concourse.bacc as bacc
    import concourse.mybir as mybir
    import concourse.tile as tile

    f32 = mybir.dt.float32
    f16 = mybir.dt.float16
    bf16 = mybir.dt.bfloat16
    Alu = mybir.AluOpType
    Act = mybir.ActivationFunctionType
    X = mybir.AxisListType.X

    ng = nt // GT                  # matmul groups
    nslab = ng // 4                # 16-tile slabs
    nblk = (nt + TPB - 1) // TPB   # w-blocks
    assert nt % TPB == 0 and ng % 4 == 0
    nact = 16 - NDVE               # ACT (complement) tiles per slab
    t_last_act = nt - NDVE - 1     # last tile with slot < nact
    t_first_dve = nact             # first tile with slot >= nact

    nc = bacc.Bacc(
        "TRN2",
        target_bir_lowering=False,
        debug=False,
        num_devices=ncores,
    )

    # ---- I/O ----
    pixT_d = nc.dram_tensor("pixT", [32, ng * P], bf16, kind="ExternalInput")
    vhwT_d = nc.dram_tensor("vhwT", [P, 2 * nt], f16, kind="ExternalInput")
    clus_d = nc.dram_tensor("clus", [C, 2], f32, kind="ExternalInput")
    chv_d = nc.dram_tensor("chv", [C, 1], f32, kind="ExternalInput")
    wB_d = nc.dram_tensor("wB", [C, nblk], f32, kind="ExternalInput")
    ident_d = nc.dram_tensor("ident", [P, P], f32, kind="ExternalInput")
    ltri_d = nc.dram_tensor("ltri", [P, P], f32, kind="ExternalInput")
    out_d = nc.dram_tensor("out", [C, 2], f32, kind="ExternalOutput")

    ACCW = 256 + 2 * nact          # acc strip: accN | accP | accT

    with tile.TileContext(nc) as tc:
        from contextlib import ExitStack

        with ExitStack() as st:
            const = st.enter_context(tc.tile_pool(name="const", bufs=1))
            stpool = st.enter_context(tc.tile_pool(name="stage", bufs=2))
            sohp = st.enter_context(tc.tile_pool(name="soh", bufs=4))
            m2p = st.enter_context(tc.tile_pool(name="m2", bufs=2))
            sqp = st.enter_context(tc.tile_pool(name="sq", bufs=2))
            recp = st.enter_context(tc.tile_pool(name="rec", bufs=2))
            vsp = st.enter_context(tc.tile_pool(name="vs", bufs=3))
            smal = st.enter_context(tc.tile_pool(name="small", bufs=4))
            eqp = st.enter_context(tc.tile_pool(name="eq", bufs=2))
            psd = st.enter_context(tc.tile_pool(name="psd", bufs=5, space="PSUM"))
            psa = st.enter_context(tc.tile_pool(name="psa", bufs=1, space="PSUM"))
            pse = st.enter_context(tc.tile_pool(name="pse", bufs=1, space="PSUM"))
            dram = st.enter_context(tc.tile_pool(name="dram", bufs=2, space="DRAM"))

            # ---- persistent SBUF state ----
            vhwT = const.tile([P, 2 * nt], f16)
            ident = const.tile([P, P], f32)
            ltri = const.tile([P, P], f32)
            chv = const.tile([C, 1], f32)
            wB = const.tile([C, nblk], f32)
            clus0 = const.tile([C, 2], f32)
            exF = const.tile([C, 8 * nblk], f32)
            rhsAll = const.tile([32, 512 * nblk], bf16)
            ones2 = const.tile([P, 2], f16)

            nc.gpsimd.dma_start(vhwT[:], vhwT_d[:])
            nc.gpsimd.dma_start(ident[:], ident_d[:])
            nc.gpsimd.dma_start(ltri[:], ltri_d[:])
            nc.gpsimd.dma_start(chv[:], chv_d[:])
            nc.gpsimd.dma_start(wB[:], wB_d[:])
            nc.gpsimd.dma_start(clus0[:], clus_d[:])

            nc.vector.memset(rhsAll[:], 0.0)
            nc.vector.memset(ones2[:], 1.0)
            for b in range(nblk):
                nc.vector.memset(exF[:, 8 * b + 4:8 * b + 6], 1.0)

            def build_ext(ncs):
                """Emit ops building rhsAll (block-diag bf16 ext) from ncs [C,2] f32."""
                # duplicate-cluster detection (first duplicate wins)
                abc = pse.tile([C, C], f32, space="PSUM", tag="bc")
                nc.tensor.transpose(
                    out=abc[:], in_=ncs[:, 0:1].to_broadcast([C, C]),
                    identity=ident[:],
                )
                eqa = eqp.tile([C, C], f32, tag="eqa")
                nc.vector.tensor_scalar(
                    out=eqa[:], in0=abc[:], scalar1=ncs[:, 0:1], scalar2=None,
                    op0=Alu.is_equal,
                )
                bbc = pse.tile([C, C], f32, space="PSUM", tag="bc")
                nc.tensor.transpose(
                    out=bbc[:], in_=ncs[:, 1:2].to_broadcast([C, C]),
                    identity=ident[:],
                )
                eqb = eqp.tile([C, C], f32, tag="eqb")
                nc.vector.tensor_scalar(
                    out=eqb[:], in0=bbc[:], scalar1=ncs[:, 1:2], scalar2=None,
                    op0=Alu.is_equal,
                )
                nc.vector.tensor_tensor(out=eqa[:], in0=eqa[:], in1=eqb[:],
                                        op=Alu.mult)
                nc.vector.tensor_tensor(out=eqa[:], in0=eqa[:], in1=ltri[:],
                                        op=Alu.mult)
                cfs = smal.tile([C, 1], f32, tag="cfs")
                nc.vector.tensor_reduce(out=cfs[:], in_=eqa[:], axis=X,
                                        op=Alu.add)

                # a-part: a' = a - ch; -2a' split hi/lo; s2base = a'^2 + BIG*dup
                aP = smal.tile([C, 1], f32, tag="aP")
                nc.vector.tensor_scalar(out=aP[:], in0=ncs[:, 0:1],
                                        scalar1=chv[:, 0:1], scalar2=None,
                                        op0=Alu.subtract)
                m2a = smal.tile([C, 1], f32, tag="m2a")
                nc.vector.tensor_scalar(out=m2a[:], in0=aP[:], scalar1=-2.0,
                                        scalar2=None, op0=Alu.mult)
                m2a_hb = smal.tile([C, 1], bf16, tag="m2a_hb")
                nc.vector.tensor_copy(out=m2a_hb[:], in_=m2a[:])
                m2a_hf = smal.tile([C, 1], f32, tag="m2a_hf")
                nc.vector.tensor_copy(out=m2a_hf[:], in_=m2a_hb[:])
                m2a_lo = smal.tile([C, 1], f32, tag="m2a_lo")
                nc.vector.tensor_tensor(out=m2a_lo[:], in0=m2a[:],
                                        in1=m2a_hf[:], op=Alu.subtract)
                s2b = smal.tile([C, 1], f32, tag="s2b")
                nc.vector.tensor_tensor(out=s2b[:], in0=aP[:], in1=aP[:],
                                        op=Alu.mult)
                nc.vector.tensor_scalar(out=cfs[:], in0=cfs[:], scalar1=BIG,
                                        scalar2=None, op0=Alu.mult)
                nc.vector.tensor_tensor(out=s2b[:], in0=s2b[:], in1=cfs[:],
                                        op=Alu.add)

                # b-part, all blocks at once: [C, nblk]
                bP8 = smal.tile([C, nblk], f32, tag="bP8")
                nc.vector.tensor_tensor(
                    out=bP8[:], in0=ncs[:, 1:2].to_broadcast([C, nblk]),
                    in1=wB[:], op=Alu.subtract)
                m2b8 = smal.tile([C, nblk], f32, tag="m2b8")
                nc.vector.tensor_scalar(out=m2b8[:], in0=bP8[:], scalar1=-2.0,
                                        scalar2=None, op0=Alu.mult)
                m2b8_hb = smal.tile([C, nblk], bf16, tag="m2b8_hb")
                nc.vector.tensor_copy(out=m2b8_hb[:], in_=m2b8[:])
                m2b8_hf = smal.tile([C, nblk], f32, tag="m2b8_hf")
                nc.vector.tensor_copy(out=m2b8_hf[:], in_=m2b8_hb[:])
                m2b8_lo = smal.tile([C, nblk], f32, tag="m2b8_lo")
                nc.vector.tensor_tensor(out=m2b8_lo[:], in0=m2b8[:],
                                        in1=m2b8_hf[:], op=Alu.subtract)
                s28 = smal.tile([C, nblk], f32, tag="s28")
                nc.vector.tensor_tensor(out=s28[:], in0=bP8[:], in1=bP8[:],
                                        op=Alu.mult)
                nc.vector.tensor_tensor(out=s28[:], in0=s28[:],
                                        in1=s2b[:].to_broadcast([C, nblk]),
                                        op=Alu.add)
                s28_hb = smal.tile([C, nblk], bf16, tag="s28_hb")
                nc.vector.tensor_copy(out=s28_hb[:], in_=s28[:])
                s28_hf = smal.tile([C, nblk], f32, tag="s28_hf")
                nc.vector.tensor_copy(out=s28_hf[:], in_=s28_hb[:])
                s28_lo = smal.tile([C, nblk], f32, tag="s28_lo")
                nc.vector.tensor_tensor(out=s28_lo[:], in0=s28[:],
                                        in1=s28_hf[:], op=Alu.subtract)

                # scatter into exF [C, (b, k)] strided views
                exV = exF[:].rearrange("p (b k) -> p b k", k=8)
                nc.vector.tensor_copy(out=exV[:, :, 0:1],
                                      in_=m2a_hf[:].to_broadcast([C, nblk, 1]))
                nc.vector.tensor_copy(out=exV[:, :, 1:2],
                                      in_=m2a_lo[:].to_broadcast([C, nblk, 1]))
                nc.vector.tensor_copy(out=exV[:, :, 2:3],
                                      in_=m2b8_hf[:].unsqueeze(2))
                nc.vector.tensor_copy(out=exV[:, :, 3:4],
                                      in_=m2b8_lo[:].unsqueeze(2))
                nc.vector.tensor_copy(out=exV[:, :, 6:7],
                                      in_=s28_hf[:].unsqueeze(2))
                nc.vector.tensor_copy(out=exV[:, :, 7:8],
                                      in_=s28_lo[:].unsqueeze(2))

                # transpose -> [8*nblk, C], cast bf16, scatter into rhsAll
                extp = pse.tile([8 * nblk, C], f32, space="PSUM", tag="extp")
                nc.tensor.transpose(out=extp[:], in_=exF[:], identity=ident[:])
                extb = smal.tile([8 * nblk, C], bf16, tag="extb")
                nc.scalar.copy(out=extb[:], in_=extp[:])
                for b in range(nblk):
                    for s in range(GT):
                        nc.gpsimd.dma_start(
                            rhsAll[8 * s:8 * s + 8,
                                   512 * b + P * s:512 * b + P * (s + 1)],
                            extb[8 * b:8 * b + 8, :])

            build_ext(clus0)

            for it in range(n_iter):
                acc = psa.tile([2, 512], f32, space="PSUM")
                stage = None
                for g in range(ng):
                    blk = (g * GT) // TPB
                    if g % CHUNK == 0:
                        stage = stpool.tile([32, CHUNK * P], bf16, tag="stage")
                        nc.gpsimd.dma_start(
                            stage[:],
                            pixT_d[:, g * P:(g + CHUNK) * P])
                    ps = psd.tile([P, GT * P], f32, space="PSUM")
                    nc.tensor.matmul(
                        out=ps[:],
                        lhsT=stage[:, (g % CHUNK) * P:(g % CHUNK + 1) * P],
                        rhs=rhsAll[:, 512 * blk:512 * (blk + 1)],
                        start=True, stop=True,
                    )

                    gg = g % 4
                    if gg == 0:
                        m2n = m2p.tile([P, 16], f32, tag="m2n")
                        slab = []
                    nc.vector.tensor_reduce(
                        out=m2n[:, gg * GT:(gg + 1) * GT],
                        in_=ps[:].rearrange("p (n x) -> p n x", x=P),
                        axis=X, op=Alu.min, negate=True,
                    )
                    slab.append(ps)
                    if gg == 3:
                        si = g // 4  # slab index
                        # ACT Sign bias: -(m2 + 0.25) so the min maps to -1,
                        # everything else to +1 (avoids Sign(0) HW semantics)
                        m2nb = m2p.tile([P, 16], f32, tag="m2nb")
                        nc.vector.tensor_scalar(
                            out=m2nb[:], in0=m2n[:], scalar1=0.25,
                            scalar2=None, op0=Alu.subtract,
                        )
                        sq = sqp.tile([P, 16], f32, tag="sq")
                        nc.vector.tensor_scalar(
                            out=sq[:], in0=m2n[:], scalar1=-1.0,
                            scalar2=1.0, op0=Alu.mult, op1=Alu.max,
                        )
                        nc.scalar.activation(out=sq[:], in_=sq[:], func=Act.Sqrt)
                        rec = recp.tile([P, 16], f16, tag="rec")
                        with nc.allow_low_precision(reason="f16 scatter weights"):
                            nc.vector.reciprocal(out=rec[:], in_=sq[:])
                        vs = vsp.tile([P, 32], f16, tag="vs")
                        nc.vector.tensor_tensor(
                            out=vs[:].rearrange("p (t d) -> p t d", d=2),
                            in0=vhwT[:, 32 * si:32 * si + 32].rearrange(
                                "p (t d) -> p t d", d=2),
                            in1=rec[:].unsqueeze(2).to_broadcast([P, 16, 2]),
                            op=Alu.mult,
                        )
                        # T: column sums of vs over the complement tiles
                        nc.tensor.matmul(
                            out=acc[:, 256:256 + 2 * nact],
                            lhsT=ones2[:],
                            rhs=vs[:, 0:2 * nact],
                            start=(si == 0), stop=(si == nslab - 1),
                        )
                        for q, psq in enumerate(slab):
                            gq = g - 3 + q
                            soh = sohp.tile([P, GT * P], f16)
                            for s in range(GT):
                                col = q * GT + s
                                t = gq * GT + s
                                sl = slice(s * P, (s + 1) * P)
                                if col >= nact:
                                    # DVE positive one-hot -> accP
                                    nc.vector.tensor_scalar(
                                        out=soh[:, sl], in0=psq[:, sl],
                                        scalar1=m2n[:, col:col + 1],
                                        scalar2=0.0,
                                        op0=Alu.add, op1=Alu.is_equal,
                                    )
                                    region = acc[:, 128:256]
                                    # bank has_written cleared once by the
                                    # slab-0 T-MM; first region write simply
                                    # overwrites (has_written=0)
                                    first = False
                                    last = (t == nt - 1)
                                else:
                                    # ACT: Sign(d2 - m2 - 0.25) in {-1,+1}
                                    # (-1 exactly at the min cluster)
                                    nc.scalar.activation(
                                        out=soh[:, sl], in_=psq[:, sl],
                                        func=Act.Sign,
                                        bias=m2nb[:, col:col + 1], scale=1.0,
                                    )
                                    region = acc[:, 0:128]
                                    first = False
                                    last = (t == t_last_act)
                                nc.tensor.matmul(
                                    out=region,
                                    lhsT=vs[:, 2 * col:2 * col + 2],
                                    rhs=soh[:, sl],
                                    start=first, stop=last,
                                )

                # ---- iteration end: AllReduce raw strip, then fixup ----
                accS = smal.tile([2, ACCW], f32, tag="accS")
                nc.scalar.copy(out=accS[:], in_=acc[:, 0:ACCW])
                arin = dram.tile([2, ACCW], f32)
                arout = dram.tile([2, ACCW], f32)
                nc.gpsimd.dma_start(arin[:], accS[:])
                nc.gpsimd.collective_compute(
                    "AllReduce",
                    Alu.add,
                    replica_groups=[list(range(ncores))],
                    ins=[arin[:].opt()],
                    outs=[arout[:].opt()],
                )
                red = smal.tile([2, ACCW], f32, tag="red")
                nc.gpsimd.dma_start(red[:], arout[:])
                # T[d] at partition d: reduce accT pairs, then diag
                t22 = smal.tile([2, 2], f32, tag="t22")
                nc.vector.tensor_reduce(
                    out=t22[:],
                    in_=red[:, 256:256 + 2 * nact].rearrange(
                        "p (t d) -> p d t", d=2),
                    axis=X, op=Alu.add,
                )
                # halve: true_sums = accP + (T - accN)/2 for the {-1,+1} form
                nc.vector.tensor_scalar(out=t22[:], in0=t22[:], scalar1=0.5,
                                        scalar2=None, op0=Alu.mult)
                tsc = smal.tile([2, 1], f32, tag="tsc")
                nc.vector.tensor_copy(out=tsc[0:1, :], in_=t22[0:1, 0:1])
                # partition base 1 is illegal for DVE; move the other diagonal
                # element with a tiny DMA instead
                nc.gpsimd.dma_start(tsc[1:2, :], t22[1:2, 1:2])
                part = smal.tile([2, C], f32, tag="part")
                nc.vector.tensor_scalar(
                    out=part[:], in0=red[:, 0:128], scalar1=-0.5,
                    scalar2=tsc[:, 0:1], op0=Alu.mult, op1=Alu.add,
                )
                nc.vector.tensor_tensor(out=part[:], in0=part[:],
                                        in1=red[:, 128:256], op=Alu.add)
                ncsp = pse.tile([C, 2], f32, space="PSUM", tag="extp")
                nc.tensor.transpose(out=ncsp[:], in_=part[:],
                                    identity=ident[0:2, 0:2])
                ncs_new = smal.tile([C, 2], f32, tag="ncs_new")
                nc.scalar.copy(out=ncs_new[:], in_=ncsp[:])

                if it == n_iter - 1:
                    nc.gpsimd.dma_start(out_d[:], ncs_new[:])
                else:
                    build_ext(ncs_new)

    nc.compile()
    return nc


def make_core_inputs(core: int, clusters: np.ndarray, heatmap: np.ndarray,
                     nt: int = NT):
    """Host-precomputed per-core constant tables."""
    import ml_dtypes
    bf16 = ml_dtypes.bfloat16

    ng = nt // GT
    nblk = max(1, nt // TPB)
    r0 = core * RPC
    ch = np.float32(r0 + 64)
    hp = (np.arange(P, dtype=np.float32) - np.float32(64.0))
    hw_h2 = hp * hp

    pixT = np.zeros((32, ng * P), np.float32)
    ts = np.arange(nt, dtype=np.float32)
    w0s = (ts // TPB) * TPB + 64.0
    wps = (ts - w0s).astype(np.float32)
    for s in range(GT):
        wp_s = wps[s::GT]                      # [ng]
        cols = pixT[:, :].reshape(32, ng, P)
        cols[8 * s + 0, :, :] = hp[None, :]
        cols[8 * s + 1, :, :] = hp[None, :]
        cols[8 * s + 2, :, :] = wp_s[:, None]
        cols[8 * s + 3, :, :] = wp_s[:, None]
        hw2 = hw_h2[None, :] + (wp_s * wp_s)[:, None]   # [ng, P]
        hi = hw2.astype(bf16).astype(np.float32)
        lo = hw2 - hi
        cols[8 * s + 4, :, :] = hi
        cols[8 * s + 5, :, :] = lo
        cols[8 * s + 6, :, :] = 1.0
        cols[8 * s + 7, :, :] = 1.0
    pixT = pixT.astype(bf16)

    hm = heatmap[r0:r0 + RPC, :nt].astype(np.float32)
    hs = (np.arange(P, dtype=np.float32) + np.float32(r0))
    vhwT = np.empty((P, 2 * nt), np.float32)
    vhwT[:, 0::2] = hs[:, None] * hm
    vhwT[:, 1::2] = np.arange(nt, dtype=np.float32)[None, :] * hm
    vhwT = vhwT.astype(np.float16)

    wB = np.broadcast_to(
        (np.arange(nblk, dtype=np.float32) * TPB + 64.0)[None, :],
        (C, nblk)).copy()

    return {
        "pixT": pixT,
        "vhwT": vhwT,
        "clus": clusters.astype(np.float32),
        "chv": np.full((C, 1), ch, np.float32),
        "wB": wB,
        "ident": np.eye(P, dtype=np.float32),
        "ltri": np.tril(np.ones((P, P), np.float32), -1),
    }


_NC_CACHE = {}


def kernel(clusters: np.ndarray, heatmap: np.ndarray) -> np.ndarray:
    _ensure_repo()
    from concourse.bass_utils import run_bass_kernel_spmd

    clusters = np.asarray(clusters, np.float32)
    heatmap = np.asarray(heatmap, np.float32)

    key = (N_ITER, NT)
    if key not in _NC_CACHE:
        _NC_CACHE[key] = build_nc()
    nc = _NC_CACHE[key]

    in_maps = [make_core_inputs(k, clusters, heatmap) for k in range(NCORES)]
    res = run_bass_kernel_spmd(nc, in_maps, list(range(NCORES)))
    return np.asarray(res.results[0]["out"], np.float32)


if __name__ == "__main__":
    _ensure_repo()
    nc = build_nc(n_iter=int(sys.argv[1]) if len(sys.argv) > 1 else 1,
                  nt=int(sys.argv[2]) if len(sys.argv) > 2 else 128)
    print("built + compiled OK")
